# revision 15
# baseline (speedup 1.0000x reference)
"""AVLnet baseline model Bass kernel for 8x TRN2 NeuronCores (v3).

Contract: kernel(**inputs) takes the FULL (unsharded) numpy inputs as produced
by setup_inputs() and returns the full (3, 32, 4096) float32 output.

Strategy vs v2:
- Conv chain fully fp16 (weights + activations). fp16 matmuls run at
  1 cyc/col at ANY free size, so the >=256-col f32r padding of conv4/conv5
  is gone: conv extents are exact per slot. PE work 1.45M -> 1.22M cols,
  conv weight DMA 64MB -> 32MB per core.
- Text/video branch weights prefetched in bulk (tpT 2.4MB, vT 0.5MB single
  DMAs) instead of 128 small per-tile DMAs: those branches were
  DMA-latency-bound and stalled the PE queue for ~90us.
- Audio tail is fully local: per-core gua GEU on its own 4 samples, then a
  local projection with the full (replicated) projT -> out2 [NS, 4096].
  Removes the AllGather of pooled audio from the tail (~18us critical path).
- Collectives kept: AG1 (t/v embeddings), AG2 (x1 transposes), AR3 (sumsq
  AllReduce) - all issued mid-conv and hidden under conv compute.
"""

import sys

for _p in ("/opt/trn_rl_repo", "/root/.axon_site/_ro/trn_rl_repo"):
    if _p not in sys.path:
        sys.path.append(_p)

from collections import deque
from contextlib import ExitStack

import numpy as np

import concourse.bass as bass
import concourse.mybir as mybir
import concourse.tile as tile
from concourse import bacc
from concourse.masks import make_identity

F32 = mybir.dt.float32
F16 = mybir.dt.float16
I32 = mybir.dt.int32
AF = mybir.ActivationFunctionType
ALU = mybir.AluOpType

NEG = -60000.0  # effectively -inf for f16 max-pool padding
NS = 4          # samples (slots) per core
NC = 8          # cores
NB = NC * NS    # total batch
RG = [list(range(NC))]  # replica group


def derive_sizes(P):
    """Per-slot conv-chain extents from pooled lengths P (each mult of 4).

    Y* = conv-out cols computed at each layer, V* = exact input cols needed.
    All values are multiples of 8; pool(Y) produces exactly V_next cols.
    """
    S = {k: [] for k in ("Y5", "V5", "Y4", "V4", "Y3", "V3", "Y2", "V2")}
    for Pa in P:
        assert 4 <= Pa <= 128 and Pa % 4 == 0
        y5 = min(256, 2 * Pa); v5 = min(256, y5 + 8)
        y4 = min(512, 2 * v5); v4 = min(512, y4 + 8)
        y3 = min(1024, 2 * v4); v3 = min(1024, y3 + 8)
        y2 = min(2048, 2 * v3); v2 = min(2048, ((y2 + 5 + 7) // 8) * 8)
        for k, v in (("Y5", y5), ("V5", v5), ("Y4", y4), ("V4", v4),
                     ("Y3", y3), ("V3", v3), ("Y2", y2), ("V2", v2)):
            S[k].append(v)
    return S


def _segs(vals, halo):
    """offsets of per-slot segments [halo | data | halo]"""
    offs, o = [], 0
    for v in vals:
        offs.append(o)
        o += v + 2 * halo
    return offs, o


def tiles_of(Y, cap=512):
    """Balanced tile sizes (multiples of 8)."""
    n = -(-Y // cap)
    base = (Y // n) // 8 * 8
    sizes = [base] * n
    rem, i = Y - base * n, 0
    while rem > 0:
        add = min(8, rem); sizes[i % n] += add; rem -= add; i += 1
    t0 = 0
    for w in sizes:
        yield t0, w
        t0 += w


def declare_io(nc, P):
    S = derive_sizes(P)
    d = {"_S": S}

    def inp(name, shape, dt):
        d[name] = nc.dram_tensor(name, list(shape), dt, kind="ExternalInput")

    # per-core data
    inp("aT", (40, sum(S["V2"])), F16)       # audio, per-slot valid prefixes
    inp("tT", (300, NS * 30), F16)           # text, (emb, sample*word)
    inp("vT", (4096, NS * 16), F16)          # video, (dim, sample*clip)
    inp("nf", (NS, 1), I32)                  # nframes//16 per slot (>=1)
    # conv weights (replicated); layout (coutp*cinp, cin128, tap*cout128).
    inp("w1T", (40, 128), F16)
    inp("b1", (128, 1), F32)
    inp("w2", (2, 128, 11 * 128), F16)
    inp("b2", (128, 2), F32)
    inp("w3", (4 * 2, 128, 17 * 128), F16)
    inp("b3", (128, 4), F32)
    inp("w4", (4 * 4, 128, 17 * 128), F16)
    inp("b4", (128, 4), F32)
    inp("w5", (8 * 4, 128, 17 * 128), F16)
    inp("b5", (128, 8), F32)
    # text branch (replicated)
    inp("tpT", (300, 4096), F16)
    inp("tpb", (128, 32), F32)
    # model-parallel GEU weight slices (per-core! each core gets its own
    # 512-wide output slice), layout (128, nk*512) k-major
    for nm in ("gutf", "gutc", "guvf", "guvc"):
        inp(nm + "T", (128, 32 * 512), F16)
        inp(nm + "b", (1, 512), F16)
    # audio GEU (1024) + full projection, replicated
    for nm in ("guaf", "guac"):
        inp(nm + "T", (128, 8 * 1024), F16)
        inp(nm + "b", (1, 1024), F16)
    inp("projT", (128, 8 * 4096), F16)
    inp("projb", (1, 4096), F16)

    d["out"] = nc.dram_tensor("out", [2, NB, 512], F32, kind="ExternalOutput")
    d["out2"] = nc.dram_tensor("out2", [NS, 4096], F32, kind="ExternalOutput")
    return d


def emit(ctx: ExitStack, tc: tile.TileContext, d, P):
    nc = tc.nc
    S = d["_S"]
    Y5, V5, Y4, V4 = S["Y5"], S["V5"], S["Y4"], S["V4"]
    Y3, V3, Y2, V2 = S["Y3"], S["V3"], S["Y2"], S["V2"]
    seg2, x2tot = _segs(V2, 5)
    seg3, x3tot = _segs(V3, 8)
    seg4, x4tot = _segs(V4, 8)
    seg5, x5tot = _segs(V5, 8)
    sega = [sum(P[:s]) for s in range(NS)]
    atot = sum(P)
    au_off = [sum(V2[:s]) for s in range(NS)]

    # ---------------- pools ----------------
    consts = ctx.enter_context(tc.tile_pool(name="consts", bufs=1))
    acts = ctx.enter_context(tc.tile_pool(name="acts", bufs=1))
    wconv = ctx.enter_context(tc.tile_pool(name="wconv", bufs=2))
    ystream = ctx.enter_context(tc.tile_pool(name="ystream", bufs=2))
    geu_sb = ctx.enter_context(tc.tile_pool(name="geu_sb", bufs=1))
    gstream = ctx.enter_context(tc.tile_pool(name="gstream", bufs=2))
    small = ctx.enter_context(tc.tile_pool(name="small", bufs=2))
    dram = ctx.enter_context(tc.tile_pool(name="dram", bufs=1, space="DRAM"))

    psum_conv = ctx.enter_context(tc.tile_pool(name="psum_conv", bufs=2, space="PSUM"))
    psum_geu = ctx.enter_context(tc.tile_pool(name="psum_geu", bufs=1, space="PSUM"))
    psum_tp = ctx.enter_context(tc.tile_pool(name="psum_tp", bufs=1, space="PSUM"))

    # ---------------- collective bounce buffers ----------------
    ag1_in = dram.tile([128, 256], F16)               # [tT16 | vT16]
    ag1_out = dram.tile([NC, 128, 2, 32, NS], F16)    # (c, i, b, k, j)
    ag2_in = dram.tile([128, 256], F16)               # [gut x1T | guv x1T]
    ag2_out = dram.tile([NC, 128, 2, 4, 32], F16)     # (c, i, b, kt, p)
    ar3_in = dram.tile([32, 2], F32)
    ar3_out = dram.tile([32, 2], F32)

    # ---------------- early bulk prefetches ----------------
    # scratch pool: early-dead prefetch tiles share the "big" tag with the
    # late-alive projw so their SBUF lifetimes overlap-free share one slot.
    scratch = ctx.enter_context(tc.tile_pool(name="scratch", bufs=1))
    # text weights: 3 chunks (300 = 128+128+44), 2.4MB total, scalar queue
    kszs = [128, 128, 44]
    tpw_all = scratch.tile([128, 3 * 4096], F16, tag="big", name="tpw_all")
    for ki, kp in enumerate(kszs):
        nc.scalar.dma_start(tpw_all[0:kp, ki * 4096:(ki + 1) * 4096],
                            d["tpT"][ki * 128: ki * 128 + kp, :])
    # video features: 512KB
    vT_sb = scratch.tile([128, 32 * NS * 16], F16, tag="vid", name="vT_sb")
    nc.gpsimd.dma_start(
        vT_sb[:].rearrange("p (c n) -> p c n", c=32),
        d["vT"][:, :].rearrange("(c p) n -> p c n", p=128))
    # full projection weights (8.4MB), replicated; needed only at the tail.
    # Filled chunk-by-chunk via the step() item machinery during conv4/5;
    # allocated lazily so the "big" slot can first serve tpw_all.
    projw_box = {}

    def get_projw():
        if "t" not in projw_box:
            projw_box["t"] = scratch.tile([128, 8 * 4096], F16, tag="big",
                                          name="projw")
        return projw_box["t"]

    # ---------------- constants ----------------
    ident = consts.tile([32, 32], F32)
    make_identity(nc, ident[:])
    ones_f = consts.tile([128, 1], F32)
    nc.vector.memset(ones_f[:], 1.0)
    ones_h = consts.tile([128, 1], F16)
    nc.vector.tensor_copy(ones_h[:], ones_f[:])
    ones_row_f = consts.tile([1, 32], F32)
    nc.vector.memset(ones_row_f[:], 1.0)
    ones_row_h = consts.tile([1, 32], F16)
    nc.vector.tensor_copy(ones_row_h[:], ones_row_f[:])

    b1t = consts.tile([128, 1], F32); nc.sync.dma_start(b1t[:], d["b1"][:, :])
    b2t = consts.tile([128, 2], F32); nc.sync.dma_start(b2t[:], d["b2"][:, :])
    b3t = consts.tile([128, 4], F32); nc.sync.dma_start(b3t[:], d["b3"][:, :])
    b4t = consts.tile([128, 4], F32); nc.sync.dma_start(b4t[:], d["b4"][:, :])
    b5t = consts.tile([128, 8], F32); nc.sync.dma_start(b5t[:], d["b5"][:, :])
    tpbt = consts.tile([128, 32], F32); nc.sync.dma_start(tpbt[:], d["tpb"][:, :])

    # ---------------- mask for audio masked-mean ----------------
    nfi = small.tile([NS, 1], I32)
    nc.sync.dma_start(nfi[:], d["nf"][:, :])
    nff = small.tile([NS, 1], F32)
    nc.vector.tensor_copy(nff[:], nfi[:])
    rnf = small.tile([NS, 1], F32)
    nc.vector.reciprocal(rnf[:], nff[:])
    iot = small.tile([NS, 128], I32)
    nc.gpsimd.iota(iot[:], pattern=[[1, 128]], base=0, channel_multiplier=0)
    iotf = small.tile([NS, 128], F32)
    nc.vector.tensor_copy(iotf[:], iot[:])
    mrow = small.tile([NS, 128], F32)
    nc.vector.tensor_scalar(mrow[:], iotf[:], nff[:], None, ALU.is_lt)
    mrow2 = small.tile([NS, 128], F32)
    nc.vector.tensor_scalar_mul(mrow2[:], mrow[:], rnf[:])
    mbs = []
    for s in range(NS):
        stage = small.tile([1, 128], F32, name=f"mstage{s}", tag="mstage")
        nc.sync.dma_start(stage[:], mrow2[s:s + 1, :])
        mb = consts.tile([128, 128], F32, name=f"mb{s}")
        nc.gpsimd.partition_broadcast(mb[:], stage[:])
        mbs.append(mb)

    # ---------------- persistent activation buffers (f16, aliased) -------
    # lifetimes: X2 [conv1,conv2], X3 [conv2,conv3], X4 [conv3,conv4],
    # X5 [conv4,conv5] -> X4 shares slot1 with X2; X5 shares slot2 with X3.
    slot1_w = max(x2tot, 4 * x4tot)
    slot2_w = max(2 * x3tot, 4 * x5tot)
    X2 = acts.tile([128, slot1_w], F16, tag="slot1")
    X3 = acts.tile([128, slot2_w], F16, tag="slot2")
    X4 = X5 = A = None  # allocated later (alias slots / tag rotation)

    def x3c(c): return X3[:, c * x3tot:(c + 1) * x3tot]
    def x4c(c): return X4[:, c * x4tot:(c + 1) * x4tot]
    def x5c(c): return X5[:, c * x5tot:(c + 1) * x5tot]
    def ac(c): return A[:, c * atot:(c + 1) * atot]

    def zero_halos(buf, segs, vals, halo, nch, tot):
        for ch in range(nch):
            for s in range(NS):
                o = ch * tot + segs[s]
                nc.vector.memset(buf[:, o:o + halo], 0.0)
                nc.vector.memset(buf[:, o + halo + vals[s]:o + 2 * halo + vals[s]], 0.0)

    zero_halos(X2, seg2, V2, 5, 1, x2tot)
    zero_halos(X3, seg3, V3, 8, 2, x3tot)

    # ---------------- conv1: (40 -> 128), k=1, relu ----------------
    w1 = consts.tile([40, 128], F16)
    nc.sync.dma_start(w1[:], d["w1T"][:, :])
    aT_sb = acts.tile([40, sum(V2)], F16, tag="A", name="aT_sb")
    nc.sync.dma_start(aT_sb[:], d["aT"][:, :])
    for s in range(NS):
        for t0, w in tiles_of(V2[s]):
            ps = psum_conv.tile([128, 512], F32, tag="cps")
            nc.tensor.matmul(ps[:, 0:w], w1[:],
                             aT_sb[:, au_off[s] + t0: au_off[s] + t0 + w],
                             start=True, stop=True)
            nc.scalar.activation(X2[:, seg2[s] + 5 + t0: seg2[s] + 5 + t0 + w],
                                 ps[:, 0:w], AF.Relu, bias=b1t[:, 0:1])

    # conv5 output buffer; rotates onto aT_sb's "A" slot (dead after conv1)
    A = acts.tile([128, 8 * atot], F32, tag="A", name="A")

    # ---------------- text branch (local samples) -> tT16 ----------------
    tT16 = geu_sb.tile([128, 128], F16)
    tTin = []
    for ki, kp in enumerate(kszs):
        t_ = consts.tile([kp, NS * 30], F16, name=f"tTin{ki}")
        nc.sync.dma_start(t_[:], d["tT"][ki * 128: ki * 128 + kp, :])
        tTin.append(t_)
    for o in range(32):
        ps = psum_conv.tile([128, NS * 30], F32, tag="cps")
        for ki, kp in enumerate(kszs):
            nc.tensor.matmul(
                ps[:], tpw_all[0:kp, ki * 4096 + o * 128: ki * 4096 + (o + 1) * 128],
                tTin[ki][:], start=(ki == 0), stop=(ki == 2))
        tw = ystream.tile([128, NS * 30], F32, tag="tw")
        nc.scalar.activation(tw[:], ps[:], AF.Relu, bias=tpbt[:, o:o + 1])
        tmax = ystream.tile([128, NS], F32, tag="tmax")
        nc.vector.reduce_max(tmax[:], tw[:].rearrange("p (s w) -> p s w", s=NS),
                             axis=mybir.AxisListType.X, opt_input=False)
        nc.vector.tensor_copy(tT16[:, o * NS:(o + 1) * NS], tmax[:])

    nc.gpsimd.dma_start(ag1_in[:, 0:128], tT16[:])

    # ---------------- video branch (local samples) -> vT16 ----------------
    vT16 = geu_sb.tile([128, 128], F16)
    vchbuf = geu_sb.tile([128, 128], F32)
    ssv_ps = psum_tp.tile([1, NS], F32, tag="tpp")
    ones_r1 = consts.tile([128, 1], F16, name="ones_r1")
    nc.vector.tensor_copy(ones_r1[:], ones_f[:])
    for c in range(32):
        vin = vT_sb[:, c * NS * 16:(c + 1) * NS * 16]
        nc.vector.reduce_max(vchbuf[:, c * NS:(c + 1) * NS],
                             vin.rearrange("p (s k) -> p s k", s=NS),
                             axis=mybir.AxisListType.X, opt_input=False)
        vsq = ystream.tile([128, NS], F16, tag="vsq")
        nc.vector.tensor_tensor(vsq[:], vchbuf[:, c * NS:(c + 1) * NS],
                                vchbuf[:, c * NS:(c + 1) * NS], ALU.mult)
        nc.tensor.matmul(ssv_ps[:], ones_r1[:], vsq[:], start=(c == 0), stop=(c == 31))
    ssv = small.tile([1, NS], F32)
    nc.vector.tensor_scalar_max(ssv[:], ssv_ps[:], 1e-24)
    ssq = small.tile([1, NS], F32)
    nc.scalar.activation(ssq[:], ssv[:], AF.Sqrt)
    ssr = small.tile([1, NS], F32)
    nc.vector.reciprocal(ssr[:], ssq[:])
    invb = consts.tile([128, NS], F32)
    nc.gpsimd.partition_broadcast(invb[:], ssr[:])
    for c in range(32):
        nc.vector.tensor_tensor(vT16[:, c * NS:(c + 1) * NS],
                                vchbuf[:, c * NS:(c + 1) * NS], invb[:], ALU.mult)

    nc.gpsimd.dma_start(ag1_in[:, 128:256], vT16[:])

    # ---------------- AG1: gather t/v chunks for all 32 samples ----------
    nc.gpsimd.collective_compute(
        "AllGather", ALU.bypass, replica_groups=RG,
        ins=[ag1_in[:].opt()], outs=[ag1_out[:].opt()])
    # readback lands core-major (8 contiguous [128, 128] blocks -> fast DMA),
    # then cheap DVE strided copies shuffle into k-major chunks [128, 32] so
    # the f-linear lhsT has a single free dim.
    tT_all = geu_sb.tile([128, 32 * 32], F16, tag="xall", bufs=2, name="tT_all")
    vT_all = geu_sb.tile([128, 32 * 32], F16, tag="xall", bufs=2, name="vT_all")
    for b, dst in ((0, tT_all), (1, vT_all)):
        cm = ystream.tile([128, 32 * 32], F16, tag="geu_tmp", name=f"cm{b}")
        for c in range(NC):
            src = ag1_out[c, :, b, :, :]  # (i, k, j) contiguous 128
            # alternate queues: these are 128x256B-line gathers (~4us each)
            eng = nc.gpsimd if c % 2 == 0 else nc.sync
            eng.dma_start(cm[:, c * 128:(c + 1) * 128], src)
        dv = dst[:].rearrange("p (k c j) -> p k c j", k=32, c=NC)
        for c in range(NC):
            nc.vector.tensor_copy(
                dv[:, :, c, :],
                cm[:, c * 128:(c + 1) * 128].rearrange("p (k j) -> p k j", k=32))

    def chunk_of(dst):
        return lambda kk: dst[:, kk * 32:(kk + 1) * 32]

    # ---------------- model-parallel GEU machinery -----------------------
    # out_slice[32, 512] = sum_k xT[k][128,32].T @ W[k][128,512]  (+ bias row)
    KI = 4

    def mp_linear_items(wkey, xT_fn, nk, epi):
        st = {}
        n_items = nk // KI

        def dma_fn(i):
            if i == 0:
                st["ps"] = psum_geu.tile([32, 512], F32, tag="gps", name="gps")
                brow = small.tile([1, 512], F16, tag="brow", name="brow")
                nc.scalar.dma_start(brow[:], d[wkey + "b"][0:1, :])
                st["brow"] = brow
            wt = gstream.tile([128, KI * 512], F16, tag="gw", name="gw")
            nc.scalar.dma_start(wt[:], d[wkey + "T"][:, i * KI * 512:(i + 1) * KI * 512])
            st[i] = wt

        def mm_fn(i):
            wt = st.pop(i)
            ps = st["ps"]
            for k in range(KI):
                kk = i * KI + k
                nc.tensor.matmul(ps[:], xT_fn(kk),
                                 wt[:, k * 512:(k + 1) * 512],
                                 start=(kk == 0), stop=False)
            if i == n_items - 1:
                brow = st.pop("brow")
                nc.tensor.matmul(ps[:], ones_row_h[:], brow[:], start=False, stop=True)
                epi(st.pop("ps"))

        for i in range(n_items):
            yield (lambda i=i: dma_fn(i)), (lambda i=i: mm_fn(i))

    class MPGeu:
        def __init__(self, name, xT_fn, fkey, ckey, out_row, ag2_col):
            self.name, self.xT_fn = name, xT_fn
            self.fkey, self.ckey = fkey, ckey
            self.out_row, self.ag2_col = out_row, ag2_col
            self.x1 = geu_sb.tile([32, 512], F32, name=f"{name}_x1")
            self.x2 = geu_sb.tile([32, 512], F32, name=f"{name}_x2")
            self.x1T_loc = geu_sb.tile([128, 128], F16, name=f"{name}_x1Tl")
            self.xcT = geu_sb.tile([128, 32 * 32], F16, tag="xall", bufs=2,
                                   name=f"{name}_xcT")

        def f_items(self):
            yield from mp_linear_items(self.fkey, self.xT_fn, 32, self.f_epi)
            yield (None, self.transpose_x1)

        def f_epi(self, ps):
            nc.scalar.copy(self.x1[:], ps[:])

        def transpose_x1(self):
            for k in range(4):
                tp = psum_tp.tile([128, 32], F32, tag="tpp")
                nc.tensor.transpose(tp[:], self.x1[:, k * 128:(k + 1) * 128],
                                    ident[0:32, 0:32])
                nc.scalar.copy(self.x1T_loc[:, k * 32:(k + 1) * 32], tp[:])
            nc.gpsimd.dma_start(ag2_in[:, self.ag2_col:self.ag2_col + 128],
                                self.x1T_loc[:])

        def c_items(self, ssb2):
            yield from mp_linear_items(
                self.ckey, lambda kk: self.xcT[:, kk * 32:(kk + 1) * 32], 32,
                lambda ps: self.c_epi(ps, ssb2))

        def c_epi(self, ps, ssb2):
            sg = ystream.tile([32, 512], F32, tag="geu_tmp", name="sg")
            nc.scalar.activation(sg[:], ps[:], AF.Sigmoid)
            nc.vector.tensor_tensor(self.x2[:], self.x1[:], sg[:], ALU.mult)
            sq = ystream.tile([32, 512], F32, tag="geu_tmp", name="sq")
            nc.scalar.activation(sq[:], self.x2[:], AF.Square,
                                 accum_out=ssb2[:, self.out_row:self.out_row + 1])

    gut = MPGeu("gut", chunk_of(tT_all), "gutf", "gutc", 0, 0)
    guv = MPGeu("guv", chunk_of(vT_all), "guvf", "guvc", 1, 128)
    ssb2 = small.tile([32, 2], F32, name="ssb2")

    def ag2_and_readback():
        nc.gpsimd.collective_compute(
            "AllGather", ALU.bypass, replica_groups=RG,
            ins=[ag2_in[:].opt()], outs=[ag2_out[:].opt()])
        for b, g in ((0, gut), (1, guv)):
            for c in range(NC):
                src = ag2_out[c, :, b, :, :]  # (i, kt, p) -> contiguous 128
                eng = nc.gpsimd if c % 2 == 0 else nc.sync
                eng.dma_start(g.xcT[:, c * 128:(c + 1) * 128], src)

    def ar3_issue():
        # issue only: the readback + scaling runs at the tail so the DVE /
        # scalar queues aren't blocked waiting on the collective mid-conv.
        nc.gpsimd.dma_start(ar3_in[:], ssb2[:])
        nc.gpsimd.collective_compute(
            "AllReduce", ALU.add, replica_groups=RG,
            ins=[ar3_in[:].opt()], outs=[ar3_out[:].opt()])

    def tv_out_epilogue():
        ssg = small.tile([32, 2], F32, name="ssg")
        nc.gpsimd.dma_start(ssg[:], ar3_out[:])
        ssm = small.tile([32, 2], F32, name="ssm")
        nc.vector.tensor_scalar_max(ssm[:], ssg[:], 1e-24)
        ssq_ = small.tile([32, 2], F32, name="ssq_")
        nc.scalar.activation(ssq_[:], ssm[:], AF.Sqrt)
        inv2 = small.tile([32, 2], F32, name="inv2")
        nc.vector.reciprocal(inv2[:], ssq_[:])
        for b, g in ((0, gut), (1, guv)):
            o_sb = ystream.tile([32, 512], F32, tag="geu_tmp", name="o_sb")
            nc.vector.tensor_scalar_mul(o_sb[:], g.x2[:], inv2[:, b:b + 1])
            nc.sync.dma_start(d["out"][b, :, :], o_sb[:])

    def projw_dma(n):
        nc.scalar.dma_start(get_projw()[:, n * 4096:(n + 1) * 4096],
                            d["projT"][:, n * 4096:(n + 1) * 4096])

    _items = deque()
    for _ in range(14):
        _items.append((None, lambda: None))
    _items.extend(gut.f_items())
    _items.extend(guv.f_items())
    _items.append((None, ag2_and_readback))
    for _ in range(9):
        _items.append((None, lambda: None))
    _items.extend(gut.c_items(ssb2))
    _items.extend(guv.c_items(ssb2))
    _items.append((None, ar3_issue))
    for n in range(8):
        _items.append(((lambda n=n: projw_dma(n)), lambda: None))
    _pending = deque()

    def step():
        if _items:
            dma_fn, mm_fn = _items.popleft()
            if dma_fn is not None:
                dma_fn()
            _pending.append(mm_fn)
            if len(_pending) > 1:
                _pending.popleft()()
        elif _pending:
            _pending.popleft()()

    def flush():
        while _items or _pending:
            step()

    # ---------------- shared conv helpers ----------------
    def maxpool_into(dst_ap, ybuf, width, tag, dt_):
        """dst[j] = max(y[2j-1],y[2j],y[2j+1]); ybuf [128, 2*width+2] padded."""
        even = ybuf[:, 0:2 * width].rearrange("p (j two) -> p j two", two=2)
        odd2 = ybuf[:, 2:2 * width + 2].rearrange("p (j two) -> p j two", two=2)
        m1 = ystream.tile([128, width], dt_, tag=tag)
        nc.vector.tensor_tensor(m1[:], even[:, :, 0], even[:, :, 1], ALU.max)
        nc.vector.tensor_tensor(dst_ap, m1[:], odd2[:, :, 0], ALU.max)

    def conv_layer(wkey, bt, n_co, n_ci, taps, xin_c, seg_in, Ys, out_fn,
                   ytag, ydt):
        """conv2/conv3 path: all ci chunks resident, per-tile psum chains."""
        halo_w = taps * 128
        for co in range(n_co):
            wts = []
            for ci in range(n_ci):
                wt = wconv.tile([128, halo_w], F16, tag=f"wc{ci}", name=f"wc{ci}")
                nc.sync.dma_start(wt[:], d[wkey][co * n_ci + ci, :, :])
                wts.append(wt)
            for s in range(NS):
                yb = ystream.tile([128, Ys[s] + 2], ydt, tag=ytag, name=ytag,
                                  bufs=1)
                nc.vector.memset(yb[:, 0:1], NEG)
                nc.vector.memset(yb[:, Ys[s] + 1:Ys[s] + 2], NEG)
                for t0, w in tiles_of(Ys[s]):
                    ps = psum_conv.tile([128, 512], F32, tag="cps", name="cps")
                    for ci in range(n_ci):
                        for tap in range(taps):
                            nc.tensor.matmul(
                                ps[:, 0:w], wts[ci][:, tap * 128:(tap + 1) * 128],
                                xin_c(ci)[:, seg_in[s] + t0 + tap: seg_in[s] + t0 + tap + w],
                                start=(ci == 0 and tap == 0),
                                stop=(ci == n_ci - 1 and tap == taps - 1))
                    nc.scalar.activation(yb[:, 1 + t0: 1 + t0 + w], ps[:, 0:w],
                                         AF.Relu, bias=bt[:, co:co + 1])
                    step()
                out_fn(co, s, yb)

    def conv_layer_pass(wkey, bt, n_co, taps, xin_c, seg_in, Cs, out_fn, ytag,
                        ydt=F32):
        """conv4/conv5 path (n_ci=4): two ci-pair passes, per-sample psum tiles
        kept alive across both passes (only 2 weight tags resident)."""
        halo_w = taps * 128
        for co in range(n_co):
            pss = [psum_conv.tile([128, Cs[s]], F32, tag=f"cp{s}", bufs=1,
                                  name=f"cp{s}") for s in range(NS)]
            for ph in range(2):
                wts = []
                for q in range(2):
                    wt = wconv.tile([128, halo_w], F16, tag=f"wc{q}", name=f"wc{q}")
                    nc.sync.dma_start(wt[:], d[wkey][co * 4 + ph * 2 + q, :, :])
                    wts.append(wt)
                for s in range(NS):
                    for q in range(2):
                        ci = ph * 2 + q
                        for tap in range(taps):
                            nc.tensor.matmul(
                                pss[s][:], wts[q][:, tap * 128:(tap + 1) * 128],
                                xin_c(ci)[:, seg_in[s] + tap: seg_in[s] + tap + Cs[s]],
                                start=(ci == 0 and tap == 0),
                                stop=(ci == 3 and tap == taps - 1))
                    step()
            for s in range(NS):
                yb = ystream.tile([128, Cs[s] + 2], ydt, tag=ytag, name=ytag)
                nc.vector.memset(yb[:, 0:1], NEG)
                nc.vector.memset(yb[:, Cs[s] + 1:Cs[s] + 2], NEG)
                nc.scalar.activation(yb[:, 1: 1 + Cs[s]], pss[s][:],
                                     AF.Relu, bias=bt[:, co:co + 1])
                step()
                out_fn(co, s, yb)

    # ---------------- conv2: 128 -> 256, k=11 ----------------
    def out2(co, s, yb):
        maxpool_into(x3c(co)[:, seg3[s] + 8: seg3[s] + 8 + Y2[s] // 2],
                     yb, Y2[s] // 2, "mp2", F16)

    conv_layer("w2", b2t, 2, 1, 11, lambda ci: X2, seg2, Y2, out2, "y2", F16)

    # ---------------- conv3: 256 -> 512, k=17 ----------------
    X4 = acts.tile([128, slot1_w], F16, tag="slot1", name="X4")
    zero_halos(X4, seg4, V4, 8, 4, x4tot)

    def out3(co, s, yb):
        maxpool_into(x4c(co)[:, seg4[s] + 8: seg4[s] + 8 + Y3[s] // 2],
                     yb, Y3[s] // 2, "mp3", F16)

    conv_layer("w3", b3t, 4, 2, 17, x3c, seg3, Y3, out3, "y3", F16)

    # ---------------- conv4: 512 -> 512, k=17 ----------------
    X5 = acts.tile([128, slot2_w], F16, tag="slot2", name="X5")
    zero_halos(X5, seg5, V5, 8, 4, x5tot)

    def out4(co, s, yb):
        maxpool_into(x5c(co)[:, seg5[s] + 8: seg5[s] + 8 + Y4[s] // 2],
                     yb, Y4[s] // 2, "mp4", F16)

    conv_layer_pass("w4", b4t, 4, 17, x4c, seg4, Y4, out4, "y4", ydt=F16)

    # ---------------- conv5: 512 -> 1024, k=17, + masked mean -------------
    xTg32 = geu_sb.tile([128, 8 * NS], F32)

    def out5(co, s, yb):
        maxpool_into(ac(co)[:, sega[s]: sega[s] + P[s]], yb, P[s], "mp5", F32)
        scr = ystream.tile([128, 128], F32, tag="mmean")
        nc.vector.scalar_tensor_tensor(
            scr[:, 0:P[s]], ac(co)[:, sega[s]: sega[s] + P[s]], 1.0,
            mbs[s][:, 0:P[s]], ALU.mult, ALU.mult,
            accum_out=xTg32[:, co * NS + s: co * NS + s + 1])

    conv_layer_pass("w5", b5t, 8, 17, x5c, seg5, Y5, out5, "y5")

    flush()
    tv_out_epilogue()

    # ---------------- audio GEU (1024, local samples) --------------------
    xTgh = geu_sb.tile([128, 8 * NS], F16)
    nc.vector.tensor_copy(xTgh[:], xTg32[:])
    ones_row_h4 = consts.tile([1, NS], F16)
    nc.vector.memset(ones_row_h4[:], 1.0)

    def gua_linear(wkey, xT, out_sb):
        pss = [psum_geu.tile([NS, 512], F32, tag="gps", name="agps0"),
               psum_tp.tile([NS, 512], F32, tag="tpp", name="agps1")]
        browa = small.tile([1, 1024], F16, tag="browa", name="browa")
        nc.scalar.dma_start(browa[:], d[wkey + "b"][0:1, :])
        for i in range(4):
            wt = gstream.tile([128, KI * 512], F16, tag="gw", name="gwa")
            nc.scalar.dma_start(wt[:], d[wkey + "T"][:, i * 2048:(i + 1) * 2048])
            for k in range(2):
                kk = i * 2 + k
                for j in range(2):
                    nc.tensor.matmul(pss[j][:], xT[:, kk * NS:(kk + 1) * NS],
                                     wt[:, k * 1024 + j * 512: k * 1024 + (j + 1) * 512],
                                     start=(kk == 0), stop=False)
        for j in range(2):
            nc.tensor.matmul(pss[j][:], ones_row_h4[:],
                             browa[:, j * 512:(j + 1) * 512], start=False, stop=True)
        for j in range(2):
            nc.scalar.copy(out_sb[:, j * 512:(j + 1) * 512], pss[j][:])

    x12a = geu_sb.tile([32 + NS, 1024], F32, name="x12a")
    x1a = x12a[0:NS, :]
    x2a_ap = x12a[32:32 + NS, :]
    gua_linear("guaf", xTgh, x1a)
    x1aT = geu_sb.tile([128, 8 * NS], F16, name="x1aT")
    for k in range(8):
        tp = psum_tp.tile([128, NS], F32, tag="tpp")
        nc.tensor.transpose(tp[:], x12a[0:NS, k * 128:(k + 1) * 128],
                            ident[0:NS, 0:NS])
        nc.scalar.copy(x1aT[:, k * NS:(k + 1) * NS], tp[:])
    g1a = ystream.tile([NS, 1024], F16, tag="gua_tmp", name="g1a")
    gua_linear("guac", x1aT, g1a)
    sga = ystream.tile([NS, 1024], F16, tag="gua_tmp", name="sga")
    nc.scalar.activation(sga[:], g1a[:], AF.Sigmoid)
    x2a = x2a_ap
    ssa = small.tile([NS, 1], F32, name="ssa")
    nc.vector.tensor_tensor(x2a, x1a, sga[:], ALU.mult)
    sqa = ystream.tile([NS, 1024], F16, tag="gua_tmp", name="sqa")
    nc.scalar.activation(sqa[:], x2a, AF.Square, accum_out=ssa[:, 0:1])
    ssam = small.tile([NS, 1], F32, name="ssam")
    nc.vector.tensor_scalar_max(ssam[:], ssa[:], 1e-24)
    ssaq = small.tile([NS, 1], F32, name="ssaq")
    nc.scalar.activation(ssaq[:], ssam[:], AF.Sqrt)
    inva = small.tile([NS, 1], F32, name="inva")
    nc.vector.reciprocal(inva[:], ssaq[:])
    ga = x1a  # x1a is dead after x2a; reuse its space for the normalized output
    nc.vector.tensor_scalar_mul(ga, x2a, inva[:, 0:1])

    # ---------------- local projection: out2[s, :] for own samples --------
    gaT = geu_sb.tile([128, 8 * NS], F16, name="gaT")
    for k in range(8):
        tp = psum_tp.tile([128, NS], F32, tag="tpp")
        nc.tensor.transpose(tp[:], x12a[0:NS, k * 128:(k + 1) * 128],
                            ident[0:NS, 0:NS])
        nc.scalar.copy(gaT[:, k * NS:(k + 1) * NS], tp[:])
    browp = small.tile([1, 4096], F16, tag="browp", name="browp")
    nc.scalar.dma_start(browp[:], d["projb"][0:1, :])
    for n in range(8):
        psp = psum_geu.tile([NS, 512], F32, tag="gps", name="pgps")
        for k in range(8):
            nc.tensor.matmul(psp[:], gaT[:, k * NS:(k + 1) * NS],
                             get_projw()[:, k * 4096 + n * 512: k * 4096 + (n + 1) * 512],
                             start=(k == 0), stop=False)
        nc.tensor.matmul(psp[:], ones_row_h4[:],
                         browp[:, n * 512:(n + 1) * 512], start=False, stop=True)
        ot_sb = ystream.tile([NS, 512], F32, tag="geu_tmp", name="ot_sb")
        nc.scalar.copy(ot_sb[:], psp[:])
        nc.sync.dma_start(d["out2"][:, n * 512:(n + 1) * 512], ot_sb[:])


def build(P):
    nc = bacc.Bacc()
    d = declare_io(nc, P)
    with tile.TileContext(nc) as tc:
        with ExitStack() as ctx:
            emit(ctx, tc, d, P)
    nc.compile()
    return nc


# ---------------------------------------------------------------------------
# host-side planning + data prep
# ---------------------------------------------------------------------------
def plan_from_inputs(inputs):
    """sample -> (core, slot) assignment and compiled slot lengths P."""
    nfr = np.asarray(inputs["audio_STFT_nframes"]).astype(np.int64)
    nf = np.maximum(1, nfr // 16)
    order = np.argsort(-nf, kind="stable")
    P = []
    for j in range(NS):
        Pa = int(nf[order[j * NC:(j + 1) * NC]].max())
        P.append(min(128, ((Pa + 3) // 4) * 4))
    return order, tuple(P)


def prep_shared(inp):
    """Replicated weights, host-transposed/cast."""
    f32, f16 = np.float32, np.float16
    w = {}
    bn_scale = (np.asarray(inp["bn_g"])[0] /
                np.sqrt(np.float32(1.0) + np.float32(1e-5))).astype(f32)
    c1 = np.asarray(inp["c1w"])[:, 0, :, 0].astype(f32)   # (128, 40)
    w["w1T"] = np.ascontiguousarray((c1 * bn_scale).T.astype(f16))
    w["b1"] = np.ascontiguousarray(
        (np.asarray(inp["c1b"]) + np.asarray(inp["bn_b"])[0] * c1.sum(1)).astype(f32)[:, None])

    def conv_w(cw, coutp, cinp, taps):
        cw = np.asarray(cw)
        ci = cw.shape[1]
        cin = ci // cinp
        a = cw[:, :, 0, :].astype(f32)                    # (Cout, Cin, taps)
        a = a.reshape(coutp, 128, cinp, cin, taps)
        a = a.transpose(0, 2, 3, 4, 1)                    # coutp, cinp, cin, tap, cout
        return np.ascontiguousarray(a.reshape(coutp * cinp, cin, taps * 128).astype(f16))

    def bias_t(b, coutp):
        return np.ascontiguousarray(np.asarray(b).astype(f32).reshape(coutp, 128).T)

    w["w2"] = conv_w(inp["c2w"], 2, 1, 11); w["b2"] = bias_t(inp["c2b"], 2)
    w["w3"] = conv_w(inp["c3w"], 4, 2, 17); w["b3"] = bias_t(inp["c3b"], 4)
    w["w4"] = conv_w(inp["c4w"], 4, 4, 17); w["b4"] = bias_t(inp["c4b"], 4)
    w["w5"] = conv_w(inp["c5w"], 8, 4, 17); w["b5"] = bias_t(inp["c5b"], 8)

    w["tpT"] = np.ascontiguousarray(np.asarray(inp["tp_w"]).astype(f32).T.astype(f16))
    w["tpb"] = np.ascontiguousarray(np.asarray(inp["tp_b"]).astype(f32).reshape(32, 128).T)

    for nm, src in (("guaf", "gua_fw"), ("guac", "gua_cw")):
        wT = np.asarray(inp[src]).astype(f32).T.astype(f16)   # (1024, 1024)
        a = wT.reshape(8, 128, 1024).transpose(1, 0, 2)
        w[nm + "T"] = np.ascontiguousarray(a.reshape(128, 8 * 1024))
        w[nm + "b"] = np.ascontiguousarray(
            np.asarray(inp[src.replace("w", "b")]).astype(f16)[None, :])

    wT = np.asarray(inp["proj_w"]).astype(f32).T.astype(f16)  # (1024, 4096)
    a = wT.reshape(8, 128, 4096).transpose(1, 0, 2)
    w["projT"] = np.ascontiguousarray(a.reshape(128, 8 * 4096))
    w["projb"] = np.ascontiguousarray(np.asarray(inp["proj_b"]).astype(f16)[None, :])
    return w


def prep_core_inputs(inp, w, order, P, core):
    """Per-core input map: local samples + this core's GEU weight slices."""
    f16 = np.float16
    S = derive_sizes(P)
    m = dict(w)
    samples = [int(order[NC * j + core]) for j in range(NS)]

    audio = np.asarray(inp["audio"]).astype(np.float32)
    m["aT"] = np.ascontiguousarray(
        np.concatenate([audio[samples[j], :, 0:S["V2"][j]] for j in range(NS)],
                       axis=1).astype(f16))
    m["tT"] = np.ascontiguousarray(
        np.asarray(inp["text"])[samples].astype(f16).transpose(2, 0, 1).reshape(300, NS * 30))
    m["vT"] = np.ascontiguousarray(
        np.asarray(inp["video"])[samples].astype(f16).transpose(2, 0, 1).reshape(4096, NS * 16))
    nfr = np.asarray(inp["audio_STFT_nframes"]).astype(np.int64)[samples]
    m["nf"] = np.ascontiguousarray(np.maximum(1, nfr // 16).astype(np.int32)[:, None])

    sl = slice(512 * core, 512 * (core + 1))
    for nm, src in (("gutf", "gut_fw"), ("gutc", "gut_cw"),
                    ("guvf", "guv_fw"), ("guvc", "guv_cw")):
        wT = np.asarray(inp[src]).astype(np.float32).T[:, sl].astype(f16)  # (4096, 512)
        a = wT.reshape(32, 128, 512).transpose(1, 0, 2)
        m[nm + "T"] = np.ascontiguousarray(a.reshape(128, 32 * 512))
        m[nm + "b"] = np.ascontiguousarray(
            np.asarray(inp[src.replace("w", "b")]).astype(f16)[None, sl])
    return m


def assemble_output(results, order):
    """results[c]: {"out": [2, 32, 512] (all samples, this core's 512 cols),
    "out2": [NS, 4096] (this core's samples, all cols)}."""
    full = np.empty((3, NB, 4096), np.float32)
    inv = np.empty(NB, np.int64)
    for p in range(NB):
        c, j = p // NS, p % NS
        inv[p] = order[NC * j + c]
    for c2 in range(NC):
        full[0:2, inv, 512 * c2:512 * (c2 + 1)] = results[c2]["out"]
        for j in range(NS):
            full[2, order[NC * j + c2], :] = results[c2]["out2"][j]
    return full


# ---------------------------------------------------------------------------
# public entry point
# ---------------------------------------------------------------------------
_NC_CACHE = {}


def _get_nc(P=None):
    if P is None:
        assert _NC_CACHE, "call kernel() or prepare() first"
        return next(iter(_NC_CACHE.values()))
    if P not in _NC_CACHE:
        _NC_CACHE[P] = build(P)
    return _NC_CACHE[P]


def prepare(inputs):
    order, P = plan_from_inputs(inputs)
    nc = _get_nc(P)
    w = prep_shared(inputs)
    in_maps = [prep_core_inputs(inputs, w, order, P, c) for c in range(NC)]
    return nc, in_maps, order, P


def kernel(**inputs):
    from concourse.bass_utils import run_bass_kernel_spmd

    nc, in_maps, order, P = prepare(inputs)
    res = run_bass_kernel_spmd(nc, in_maps, core_ids=list(range(NC)))
    return assemble_output([res.results[c] for c in range(NC)], order)


# revision 39
# speedup vs baseline: 4.6424x; 4.6424x over previous
"""AVLnet baseline model Bass kernel for 8x TRN2 NeuronCores (v3).

Contract: kernel(**inputs) takes the FULL (unsharded) numpy inputs as produced
by setup_inputs() and returns the full (3, 32, 4096) float32 output.

Strategy vs v2:
- Conv chain fully fp16 (weights + activations). fp16 matmuls run at
  1 cyc/col at ANY free size, so the >=256-col f32r padding of conv4/conv5
  is gone: conv extents are exact per slot. PE work 1.45M -> 1.22M cols,
  conv weight DMA 64MB -> 32MB per core.
- Text/video branch weights prefetched in bulk (tpT 2.4MB, vT 0.5MB single
  DMAs) instead of 128 small per-tile DMAs: those branches were
  DMA-latency-bound and stalled the PE queue for ~90us.
- Audio tail is fully local: per-core gua GEU on its own 4 samples, then a
  local projection with the full (replicated) projT -> out2 [NS, 4096].
  Removes the AllGather of pooled audio from the tail (~18us critical path).
- Collectives kept: AG1 (t/v embeddings), AG2 (x1 transposes), AR3 (sumsq
  AllReduce) - all issued mid-conv and hidden under conv compute.
"""

import sys

for _p in ("/opt/trn_rl_repo", "/root/.axon_site/_ro/trn_rl_repo"):
    if _p not in sys.path:
        sys.path.append(_p)

from collections import deque
from contextlib import ExitStack

import numpy as np

import concourse.bass as bass
import concourse.mybir as mybir
import concourse.tile as tile
from concourse import bacc
from concourse.masks import make_identity

F32 = mybir.dt.float32
F16 = mybir.dt.float16
I32 = mybir.dt.int32
AF = mybir.ActivationFunctionType
ALU = mybir.AluOpType

NEG = -60000.0  # effectively -inf for f16 max-pool padding
NS = 4          # samples (slots) per core
NC = 8          # cores
NB = NC * NS    # total batch
RG = [list(range(NC))]  # replica group


def derive_sizes(P):
    """Per-slot conv-chain extents from pooled lengths P (each mult of 4).

    Y* = conv-out cols computed at each layer, V* = exact input cols needed.
    All values are multiples of 8; pool(Y) produces exactly V_next cols.
    """
    S = {k: [] for k in ("Y5", "V5", "Y4", "V4", "Y3", "V3", "Y2", "V2")}
    for Pa in P:
        assert 4 <= Pa <= 128 and Pa % 4 == 0
        y5 = min(256, 2 * Pa); v5 = min(256, y5 + 8)
        y4 = min(512, 2 * v5); v4 = min(512, y4 + 8)
        y3 = min(1024, 2 * v4); v3 = min(1024, y3 + 8)
        y2 = min(2048, 2 * v3); v2 = min(2048, ((y2 + 5 + 7) // 8) * 8)
        for k, v in (("Y5", y5), ("V5", v5), ("Y4", y4), ("V4", v4),
                     ("Y3", y3), ("V3", v3), ("Y2", y2), ("V2", v2)):
            S[k].append(v)
    return S


def _segs(vals, halo):
    """offsets of per-slot segments [halo | data | halo]"""
    offs, o = [], 0
    for v in vals:
        offs.append(o)
        o += v + 2 * halo
    return offs, o


def tiles_of(Y, cap=512):
    """Balanced tile sizes (multiples of 8)."""
    n = -(-Y // cap)
    base = (Y // n) // 8 * 8
    sizes = [base] * n
    rem, i = Y - base * n, 0
    while rem > 0:
        add = min(8, rem); sizes[i % n] += add; rem -= add; i += 1
    t0 = 0
    for w in sizes:
        yield t0, w
        t0 += w


def declare_io(nc, P):
    S = derive_sizes(P)
    d = {"_S": S}

    def inp(name, shape, dt):
        d[name] = nc.dram_tensor(name, list(shape), dt, kind="ExternalInput")

    # per-core data
    inp("aT", (40, sum(S["V2"])), F16)       # audio, per-slot valid prefixes
    inp("tT", (300, NS * 30), F16)           # text, (emb, sample*word)
    inp("vT", (4096, NS * 16), F16)          # video, (dim, sample*clip)
    inp("nf", (NS, 1), I32)                  # nframes//16 per slot (>=1)
    # conv weights (replicated); layout (coutp*cinp, cin128, tap*cout128).
    inp("w1T", (40, 128), F16)
    inp("b1", (128, 1), F32)
    inp("w2", (2, 128, 11 * 128), F16)
    inp("b2", (128, 2), F32)
    inp("w3", (4 * 2, 128, 17 * 128), F16)
    inp("b3", (128, 4), F32)
    inp("w4", (4 * 4, 128, 17 * 128), F16)
    inp("b4", (128, 4), F32)
    inp("w5", (8 * 4, 128, 17 * 128), F16)
    inp("b5", (128, 8), F32)
    # text branch (replicated)
    inp("tpT", (300, 4096), F16)
    inp("tpb", (128, 32), F32)
    # model-parallel GEU weight slices (per-core! each core gets its own
    # 512-wide output slice), layout (128, nk*512) k-major
    for nm in ("gutf", "gutc", "guvf", "guvc"):
        inp(nm + "T", (128, 32 * 512), F16)
        inp(nm + "b", (1, 512), F16)
    # audio GEU (1024) + full projection, replicated
    for nm in ("guaf", "guac"):
        inp(nm + "T", (128, 8 * 1024), F16)
        inp(nm + "b", (1, 1024), F16)
    inp("projT", (128, 8 * 4096), F16)
    inp("projb", (1, 4096), F16)

    d["out"] = nc.dram_tensor("out", [2, NB, 512], F32, kind="ExternalOutput")
    d["out2"] = nc.dram_tensor("out2", [NS, 4096], F32, kind="ExternalOutput")
    return d


def emit(ctx: ExitStack, tc: tile.TileContext, d, P):
    nc = tc.nc
    S = d["_S"]
    Y5, V5, Y4, V4 = S["Y5"], S["V5"], S["Y4"], S["V4"]
    Y3, V3, Y2, V2 = S["Y3"], S["V3"], S["Y2"], S["V2"]
    seg2, x2tot = _segs(V2, 5)
    seg3, x3tot = _segs(V3, 8)
    seg4, x4tot = _segs(V4, 8)
    seg5, x5tot = _segs(V5, 8)
    sega = [sum(P[:s]) for s in range(NS)]
    atot = sum(P)
    au_off = [sum(V2[:s]) for s in range(NS)]

    # ---------------- pools ----------------
    consts = ctx.enter_context(tc.tile_pool(name="consts", bufs=1))
    acts = ctx.enter_context(tc.tile_pool(name="acts", bufs=1))
    wconv = ctx.enter_context(tc.tile_pool(name="wconv", bufs=2))
    ystream = ctx.enter_context(tc.tile_pool(name="ystream", bufs=2))
    geu_sb = ctx.enter_context(tc.tile_pool(name="geu_sb", bufs=1))
    gstream = ctx.enter_context(tc.tile_pool(name="gstream", bufs=2))
    small = ctx.enter_context(tc.tile_pool(name="small", bufs=2))
    dram = ctx.enter_context(tc.tile_pool(name="dram", bufs=1, space="DRAM"))

    psum_conv = ctx.enter_context(tc.tile_pool(name="psum_conv", bufs=2, space="PSUM"))
    psum_geu = ctx.enter_context(tc.tile_pool(name="psum_geu", bufs=1, space="PSUM"))
    psum_tp = ctx.enter_context(tc.tile_pool(name="psum_tp", bufs=1, space="PSUM"))

    # ---------------- collective bounce buffers ----------------
    ag1_in = dram.tile([128, 256], F16)               # [tT16 | vT16]
    ag1_out = dram.tile([NC, 128, 2, 32, NS], F16)    # (c, i, b, k, j)
    ag2_in = dram.tile([128, 256], F16)               # [gut x1T | guv x1T]
    ag2_out = dram.tile([NC, 128, 2, 4, 32], F16)     # (c, i, b, kt, p)
    ar3_in = dram.tile([32, 2], F32)
    ar3_out = dram.tile([32, 2], F32)

    # ---------------- early bulk prefetches ----------------
    # conv1 inputs first on sync so PE starts ASAP
    w1 = consts.tile([40, 128], F16)
    nc.sync.dma_start(w1[:], d["w1T"][:, :])
    b1t = consts.tile([128, 1], F32)
    nc.sync.dma_start(b1t[:], d["b1"][:, :])
    aT_sb = acts.tile([40, sum(V2)], F16, tag="A", name="aT_sb")
    for s in range(NS):
        eng = nc.sync if s % 2 == 0 else nc.scalar
        eng.dma_start(aT_sb[:, au_off[s]:au_off[s] + V2[s]],
                      d["aT"][:, au_off[s]:au_off[s] + V2[s]])
    # scratch pool: early-dead prefetch tiles share the "big" tag with the
    # late-alive projw so their SBUF lifetimes overlap-free share one slot.
    scratch = ctx.enter_context(tc.tile_pool(name="scratch", bufs=1))
    # text weights: 3 chunks (300 = 128+128+44), 2.4MB, gpsimd queue
    kszs = [128, 128, 44]
    tpw_all = scratch.tile([128, 3 * 4096], F16, tag="big", name="tpw_all")
    for ki, kp in enumerate(kszs):
        nc.gpsimd.dma_start(tpw_all[0:kp, ki * 4096:(ki + 1) * 4096],
                            d["tpT"][ki * 128: ki * 128 + kp, :])
    # video features: 512KB
    vT_sb = scratch.tile([128, 32 * NS * 16], F16, tag="vid", name="vT_sb")
    nc.gpsimd.dma_start(
        vT_sb[:].rearrange("p (c n) -> p c n", c=32),
        d["vT"][:, :].rearrange("(c p) n -> p c n", p=128))
    # full projection weights (8.4MB), replicated; needed only at the tail.
    # Filled chunk-by-chunk via the step() item machinery during conv4/5;
    # allocated lazily so the "big" slot can first serve tpw_all.
    projw_box = {}

    def get_projw():
        if "t" not in projw_box:
            projw_box["t"] = scratch.tile([128, 8 * 4096], F16, tag="big",
                                          name="projw")
        return projw_box["t"]

    # ---------------- constants ----------------
    ident = consts.tile([32, 32], F32)
    make_identity(nc, ident[:])
    ones_f = consts.tile([128, 1], F32)
    nc.vector.memset(ones_f[:], 1.0)
    ones_h = consts.tile([128, 1], F16)
    nc.vector.tensor_copy(ones_h[:], ones_f[:])
    ones_row_f = consts.tile([1, 32], F32)
    nc.vector.memset(ones_row_f[:], 1.0)
    ones_row_h = consts.tile([1, 32], F16)
    nc.vector.tensor_copy(ones_row_h[:], ones_row_f[:])

    b2t = consts.tile([128, 2], F32); nc.scalar.dma_start(b2t[:], d["b2"][:, :])
    b3t = consts.tile([128, 4], F32); nc.scalar.dma_start(b3t[:], d["b3"][:, :])
    b4t = consts.tile([128, 4], F32); nc.scalar.dma_start(b4t[:], d["b4"][:, :])
    b5t = consts.tile([128, 8], F32); nc.scalar.dma_start(b5t[:], d["b5"][:, :])
    tpbt = consts.tile([128, 32], F32)
    nc.scalar.dma_start(tpbt[:], d["tpb"][:, :])

    # ---------------- mask for audio masked-mean ----------------
    nfi = small.tile([NS, 1], I32)
    nc.scalar.dma_start(nfi[:], d["nf"][:, :])
    nff = small.tile([NS, 1], F32)
    nc.vector.tensor_copy(nff[:], nfi[:])
    rnf = small.tile([NS, 1], F32)
    nc.vector.reciprocal(rnf[:], nff[:])
    iot = small.tile([NS, 128], I32)
    nc.gpsimd.iota(iot[:], pattern=[[1, 128]], base=0, channel_multiplier=0)
    iotf = small.tile([NS, 128], F32)
    nc.vector.tensor_copy(iotf[:], iot[:])
    mrow = small.tile([NS, 128], F32)
    nc.vector.tensor_scalar(mrow[:], iotf[:], nff[:], None, ALU.is_lt)
    mrow2 = small.tile([NS, 128], F32)
    nc.vector.tensor_scalar_mul(mrow2[:], mrow[:], rnf[:])
    mrow2h = small.tile([NS, 128], F16, name="mrow2h")
    nc.vector.tensor_copy(mrow2h[:], mrow2[:])
    mbs = []
    for s in range(NS):
        stage = small.tile([1, 128], F16, name=f"mstage{s}", tag="mstage")
        nc.scalar.dma_start(stage[:], mrow2h[s:s + 1, :])
        mb = consts.tile([128, 128], F16, name=f"mb{s}")
        nc.gpsimd.partition_broadcast(mb[:], stage[:])
        mbs.append(mb)

    # ---------------- persistent activation buffers (f16, aliased) -------
    # lifetimes: X2 [conv1,conv2], X3 [conv2,conv3], X4 [conv3,conv4],
    # X5 [conv4,conv5] -> X4 shares slot1 with X2; X5 shares slot2 with X3.
    slot1_w = max(x2tot, 4 * x4tot)
    slot2_w = max(2 * x3tot, 4 * x5tot)
    X2 = acts.tile([128, slot1_w], F16, tag="slot1")
    X3 = acts.tile([128, slot2_w], F16, tag="slot2")
    X4 = X5 = A = None  # allocated later (alias slots / tag rotation)

    def x3c(c): return X3[:, c * x3tot:(c + 1) * x3tot]
    def x4c(c): return X4[:, c * x4tot:(c + 1) * x4tot]
    def x5c(c): return X5[:, c * x5tot:(c + 1) * x5tot]
    def ac(c): return A[:, c * atot:(c + 1) * atot]

    def zero_halos(buf, segs, vals, halo, nch, tot):
        for ch in range(nch):
            for s in range(NS):
                o = ch * tot + segs[s]
                nc.vector.memset(buf[:, o:o + halo], 0.0)
                nc.vector.memset(buf[:, o + halo + vals[s]:o + 2 * halo + vals[s]], 0.0)

    zero_halos(X2, seg2, V2, 5, 1, x2tot)
    zero_halos(X3, seg3, V3, 8, 2, x3tot)

    # ---------------- conv1: (40 -> 128), k=1, relu ----------------
    for s in range(NS):
        for t0, w in tiles_of(V2[s]):
            ps = psum_conv.tile([128, 512], F32, tag="cps")
            nc.tensor.matmul(ps[:, 0:w], w1[:],
                             aT_sb[:, au_off[s] + t0: au_off[s] + t0 + w],
                             start=True, stop=True)
            nc.scalar.activation(X2[:, seg2[s] + 5 + t0: seg2[s] + 5 + t0 + w],
                                 ps[:, 0:w], AF.Relu, bias=b1t[:, 0:1])

    # conv5 output buffer; rotates onto aT_sb's "A" slot (dead after conv1)
    A = acts.tile([128, 8 * atot], F32, tag="A", name="A")

    # ---------------- video branch part 1: channel max + squares -------
    # emitted before the text loop so the big DVE reduce runs while the
    # text matmuls stream (AG1 critical path)
    vT16 = geu_sb.tile([128, 128], F16)
    vchbuf = geu_sb.tile([128, 128], F32)
    ones_r1 = consts.tile([128, 1], F16, name="ones_r1")
    nc.vector.tensor_copy(ones_r1[:], ones_f[:])
    vv = vT_sb[:].rearrange("p (c s k) -> p (c s) k", c=32, s=NS)

    def video_reduce_part(q):
        nc.vector.reduce_max(vchbuf[:, q * 32:(q + 1) * 32],
                             vv[:, q * 32:(q + 1) * 32, :],
                             axis=mybir.AxisListType.X, opt_input=False)

    # ---------------- text branch (local samples) -> tT16 ----------------
    # bias + relu fused into the word-max epilogue on DVE (max and +bias
    # commute, relu is monotone) so no per-o Activation op.
    tT16 = geu_sb.tile([128, 128], F16)
    tTin = []
    for ki, kp in enumerate(kszs):
        t_ = consts.tile([kp, NS * 30], F16, name=f"tTin{ki}")
        nc.sync.dma_start(t_[:], d["tT"][ki * 128: ki * 128 + kp, :])
        tTin.append(t_)
    for ob in range(8):
        if ob < 4:
            video_reduce_part(ob)
        ps = psum_conv.tile([128, 4 * NS * 30], F32, tag="cps")
        for oi in range(4):
            o = ob * 4 + oi
            for ki, kp in enumerate(kszs):
                nc.tensor.matmul(
                    ps[:, oi * 120:(oi + 1) * 120],
                    tpw_all[0:kp, ki * 4096 + o * 128: ki * 4096 + (o + 1) * 128],
                    tTin[ki][:], start=(ki == 0), stop=(ki == 2))
        tmax = ystream.tile([128, 4 * NS], F32, tag="tmax")
        nc.vector.reduce_max(tmax[:], ps[:].rearrange("p (o s w) -> p (o s) w", o=4, s=NS),
                             axis=mybir.AxisListType.X, opt_input=False)
        for oi in range(4):
            o = ob * 4 + oi
            nc.vector.tensor_scalar(tT16[:, o * NS:(o + 1) * NS],
                                    tmax[:, oi * NS:(oi + 1) * NS],
                                    tpbt[:, o:o + 1], 0.0, ALU.add, ALU.max)

    nc.gpsimd.dma_start(ag1_in[:, 0:128], tT16[:])

    # ---------------- video branch part 2: normalize -> vT16 ----------
    vsq = ystream.tile([128, 128], F16, tag="vsq")
    nc.vector.tensor_tensor(vsq[:], vchbuf[:], vchbuf[:], ALU.mult)
    ssv_ps = psum_tp.tile([1, 128], F32, tag="tpp")
    nc.tensor.matmul(ssv_ps[:], ones_r1[:], vsq[:], start=True, stop=True)
    ssv = small.tile([1, NS], F32)
    nc.vector.reduce_sum(ssv[:], ssv_ps[:].rearrange("one (c j) -> one j c", j=NS),
                         axis=mybir.AxisListType.X, opt_input=False)
    nc.vector.tensor_scalar_max(ssv[:], ssv[:], 1e-24)
    ssq = small.tile([1, NS], F32)
    nc.scalar.activation(ssq[:], ssv[:], AF.Sqrt)
    ssr = small.tile([1, 4 * NS], F32, name="ssr")
    nc.vector.reciprocal(ssr[:, 0:NS], ssq[:])
    for r in (1, 2):
        nc.vector.tensor_copy(ssr[:, r * NS:2 * r * NS], ssr[:, 0:r * NS])
    invb = consts.tile([128, 4 * NS], F32)
    nc.gpsimd.partition_broadcast(invb[:], ssr[:])
    for c in range(8):
        nc.vector.tensor_tensor(vT16[:, c * 16:(c + 1) * 16],
                                vchbuf[:, c * 16:(c + 1) * 16], invb[:], ALU.mult)

    nc.gpsimd.dma_start(ag1_in[:, 128:256], vT16[:])

    # ---------------- AG1: gather t/v chunks for all 32 samples ----------
    nc.gpsimd.collective_compute(
        "AllGather", ALU.bypass, replica_groups=RG,
        ins=[ag1_in[:].opt()], outs=[ag1_out[:].opt()])
    # readback lands core-major (8 contiguous [128, 128] blocks -> fast DMA),
    # then cheap DVE strided copies shuffle into k-major chunks [128, 32] so
    # the f-linear lhsT has a single free dim.
    # landed core-major [p, c*128 + k*4 + j]; DVE shuffle into k-major
    # chunks, scheduled via wait hints so the copies sit late in the DVE
    # queue (they depend on the 28us collective).
    tT_all = geu_sb.tile([128, 32 * 32], F16, tag="xall", bufs=2, name="tT_all")
    vT_all = geu_sb.tile([128, 32 * 32], F16, tag="xall", bufs=2, name="vT_all")
    for b, dst in ((0, tT_all), (1, vT_all)):
        cm = ystream.tile([128, 32 * 32], F16, tag="geu_tmp", name=f"cm{b}")
        for c in range(NC):
            src = ag1_out[c, :, b, :, :]  # (i, k, j) contiguous 128
            # gpsimd ring only: these wait on the collective, and any other
            # ring would block its later (conv-weight) transfers behind them
            nc.gpsimd.dma_start(cm[:, c * 128:(c + 1) * 128], src)
        dv = dst[:].rearrange("p (k c j) -> p k c j", k=32, c=NC)
        with tc.tile_wait_until(0.057 + 0.004 * b):
            for c in range(NC):
                nc.vector.tensor_copy(
                    dv[:, :, c, :],
                    cm[:, c * 128:(c + 1) * 128].rearrange("p (k j) -> p k j", k=32))

    def chunk_of(dst):
        return lambda kk: dst[:, kk * 32:(kk + 1) * 32]

    # ---------------- model-parallel GEU machinery -----------------------
    # out_slice[32, 512] = sum_k xT[k][128,32].T @ W[k][128,512]  (+ bias row)
    KI = 4

    def mp_linear_items(wkey, xT_fn, nk, epi, wait_ms):
        st = {}
        n_items = nk // KI

        def dma_fn(i):
            if i == 0:
                st["ps"] = psum_geu.tile([32, 512], F32, tag="gps", name="gps")
                brow = small.tile([1, 512], F16, tag="brow", name="brow")
                nc.scalar.dma_start(brow[:], d[wkey + "b"][0:1, :])
                st["brow"] = brow
            wt = gstream.tile([128, KI * 512], F16, tag="gw", name="gw")
            nc.scalar.dma_start(wt[:], d[wkey + "T"][:, i * KI * 512:(i + 1) * KI * 512])
            st[i] = wt

        def mm_fn(i):
            # the tile scheduler models collectives as ~free; the wait hint
            # keeps these matmuls from being slotted before their gathered
            # operand can really exist
            with tc.tile_wait_until(wait_ms + i * 0.0015):
                wt = st.pop(i)
                ps = st["ps"]
                for k in range(KI):
                    kk = i * KI + k
                    nc.tensor.matmul(ps[:], xT_fn(kk),
                                     wt[:, k * 512:(k + 1) * 512],
                                     start=(kk == 0), stop=False)
                if i == n_items - 1:
                    brow = st.pop("brow")
                    nc.tensor.matmul(ps[:], ones_row_h[:], brow[:],
                                     start=False, stop=True)
                    epi(st.pop("ps"))

        for i in range(n_items):
            yield (lambda i=i: dma_fn(i)), (lambda i=i: mm_fn(i))

    class MPGeu:
        def __init__(self, name, xT_fn, fkey, ckey, out_row, ag2_col,
                     f_wait, c_wait):
            self.name, self.xT_fn = name, xT_fn
            self.f_wait, self.c_wait = f_wait, c_wait
            self.fkey, self.ckey = fkey, ckey
            self.out_row, self.ag2_col = out_row, ag2_col
            self.x1 = geu_sb.tile([32, 512], F32, name=f"{name}_x1")
            self.x2 = geu_sb.tile([32, 512], F32, name=f"{name}_x2")
            self.x1T_loc = geu_sb.tile([128, 128], F16, name=f"{name}_x1Tl")
            self.xcT = geu_sb.tile([128, 32 * 32], F16, tag="xall", bufs=2,
                                   name=f"{name}_xcT")

        def f_items(self):
            yield from mp_linear_items(self.fkey, self.xT_fn, 32, self.f_epi,
                                       self.f_wait)
            yield (None, self.transpose_x1)

        def f_epi(self, ps):
            nc.scalar.copy(self.x1[:], ps[:])

        def transpose_x1(self):
          with tc.tile_wait_until(self.f_wait + 0.014):
            for k in range(4):
                tp = psum_tp.tile([128, 32], F32, tag="tpp")
                nc.tensor.transpose(tp[:], self.x1[:, k * 128:(k + 1) * 128],
                                    ident[0:32, 0:32])
                nc.scalar.copy(self.x1T_loc[:, k * 32:(k + 1) * 32], tp[:])
            nc.gpsimd.dma_start(ag2_in[:, self.ag2_col:self.ag2_col + 128],
                                self.x1T_loc[:])

        def c_items(self, ssb2):
            yield from mp_linear_items(
                self.ckey, lambda kk: self.xcT[:, kk * 32:(kk + 1) * 32], 32,
                lambda ps: self.c_epi(ps, ssb2), self.c_wait)

        def c_epi(self, ps, ssb2):
            sg = ystream.tile([32, 512], F32, tag="geu_tmp", name="sg")
            nc.scalar.activation(sg[:], ps[:], AF.Sigmoid)
            nc.vector.tensor_tensor(self.x2[:], self.x1[:], sg[:], ALU.mult)
            sq = ystream.tile([32, 512], F32, tag="geu_tmp", name="sq")
            nc.scalar.activation(sq[:], self.x2[:], AF.Square,
                                 accum_out=ssb2[:, self.out_row:self.out_row + 1])

    gut = MPGeu("gut", chunk_of(tT_all), "gutf", "gutc", 0, 0, 0.062, 0.141)
    guv = MPGeu("guv", chunk_of(vT_all), "guvf", "guvc", 1, 128, 0.066, 0.150)
    ssb2 = small.tile([32, 2], F32, name="ssb2")

    def ag2_and_readback():
        nc.gpsimd.collective_compute(
            "AllGather", ALU.bypass, replica_groups=RG,
            ins=[ag2_in[:].opt()], outs=[ag2_out[:].opt()])
        for b, g in ((0, gut), (1, guv)):
            for c in range(NC):
                src = ag2_out[c, :, b, :, :]  # (i, kt, p) -> contiguous 128
                nc.gpsimd.dma_start(g.xcT[:, c * 128:(c + 1) * 128], src)

    def ar3_issue():
        # issue only: the readback + scaling runs at the tail so the DVE /
        # scalar queues aren't blocked waiting on the collective mid-conv.
        nc.gpsimd.dma_start(ar3_in[:], ssb2[:])
        nc.gpsimd.collective_compute(
            "AllReduce", ALU.add, replica_groups=RG,
            ins=[ar3_in[:].opt()], outs=[ar3_out[:].opt()])

    def tv_out_epilogue():
        ssg = small.tile([32, 2], F32, name="ssg")
        nc.gpsimd.dma_start(ssg[:], ar3_out[:])
        ssm = small.tile([32, 2], F32, name="ssm")
        nc.vector.tensor_scalar_max(ssm[:], ssg[:], 1e-24)
        ssq_ = small.tile([32, 2], F32, name="ssq_")
        nc.scalar.activation(ssq_[:], ssm[:], AF.Sqrt)
        inv2 = small.tile([32, 2], F32, name="inv2")
        nc.vector.reciprocal(inv2[:], ssq_[:])
        for b, g in ((0, gut), (1, guv)):
            o_sb = ystream.tile([32, 512], F32, tag="geu_tmp", name="o_sb")
            nc.vector.tensor_scalar_mul(o_sb[:], g.x2[:], inv2[:, b:b + 1])
            nc.sync.dma_start(d["out"][b, :, :], o_sb[:])

    def projw_dma(n):
        nc.scalar.dma_start(get_projw()[:, n * 4096:(n + 1) * 4096],
                            d["projT"][:, n * 4096:(n + 1) * 4096])

    _items = deque()
    for _ in range(2):
        _items.append((None, lambda: None))
    _items.extend(gut.f_items())
    _items.extend(guv.f_items())
    _items.append((None, ag2_and_readback))
    for _ in range(9):
        _items.append((None, lambda: None))
    _items.extend(gut.c_items(ssb2))
    _items.extend(guv.c_items(ssb2))
    _items.append((None, ar3_issue))
    for n in range(8):
        _items.append(((lambda n=n: projw_dma(n)), lambda: None))

    guaf_box, guac_box = {}, {}

    def gua_chunk_dma(box, wkey, i):
        # guac chunks get a dedicated 4-deep tag so none of them waits on
        # the rotation of still-live guaf buffers
        tag, bufs = ("gw", 2) if wkey == "guaf" else ("gwc", 4)
        wt = gstream.tile([128, KI * 512], F16, tag=tag, bufs=bufs,
                          name=f"{wkey}{i}")
        nc.gpsimd.dma_start(wt[:], d[wkey + "T"][:, i * 2048:(i + 1) * 2048])
        box[i] = wt

    for i in range(2):
        _items.append(((lambda i=i: gua_chunk_dma(guaf_box, "guaf", i)),
                       lambda: None))
    _pending = deque()

    def step():
        if _items:
            dma_fn, mm_fn = _items.popleft()
            if dma_fn is not None:
                dma_fn()
            _pending.append(mm_fn)
            if len(_pending) > 1:
                _pending.popleft()()
        elif _pending:
            _pending.popleft()()

    def flush():
        while _items or _pending:
            step()

    # ---------------- shared conv helpers ----------------
    def maxpool_into(dst_ap, ybuf, width, tag, dt_):
        """dst[j] = max(y[2j-1],y[2j],y[2j+1]); ybuf [128, 2*width+2] padded."""
        even = ybuf[:, 0:2 * width].rearrange("p (j two) -> p j two", two=2)
        odd2 = ybuf[:, 2:2 * width + 2].rearrange("p (j two) -> p j two", two=2)
        m1 = ystream.tile([128, width], dt_, tag=tag, bufs=1)
        nc.vector.tensor_tensor(m1[:], even[:, :, 0], even[:, :, 1], ALU.max)
        nc.vector.tensor_tensor(dst_ap, m1[:], odd2[:, :, 0], ALU.max)

    def conv_layer(wkey, bt, n_co, n_ci, taps, xin_c, seg_in, Ys, out_fn,
                   ytag, ydt):
        """conv2/conv3 path: all ci chunks resident, per-tile psum chains.
        Pooling runs per tile on a rolling [128, 514] buffer (col 0 carries
        the previous tile's last y column)."""
        halo_w = taps * 128
        for co in range(n_co):
            wts = []
            for ci in range(n_ci):
                wt = wconv.tile([128, halo_w], F16, tag=f"wc{ci}", name=f"wc{ci}")
                nc.sync.dma_start(wt[:], d[wkey][co * n_ci + ci, :, :])
                wts.append(wt)
            for s in range(NS):
                prev = None
                for t0, w in tiles_of(Ys[s]):
                    yb = ystream.tile([128, 514], ydt, tag=ytag, name=ytag)
                    if prev is None:
                        nc.vector.memset(yb[:, 0:1], NEG)
                    else:
                        nc.vector.tensor_copy(yb[:, 0:1], prev)
                    ps = psum_conv.tile([128, 512], F32, tag="cps", name="cps")
                    for ci in range(n_ci):
                        for tap in range(taps):
                            nc.tensor.matmul(
                                ps[:, 0:w], wts[ci][:, tap * 128:(tap + 1) * 128],
                                xin_c(ci)[:, seg_in[s] + t0 + tap: seg_in[s] + t0 + tap + w],
                                start=(ci == 0 and tap == 0),
                                stop=(ci == n_ci - 1 and tap == taps - 1))
                    nc.scalar.activation(yb[:, 1: 1 + w], ps[:, 0:w],
                                         AF.Relu, bias=bt[:, co:co + 1])
                    step()
                    if t0 + w == Ys[s]:
                        nc.vector.memset(yb[:, w + 1:w + 2], NEG)
                    prev = yb[:, w:w + 1]
                    out_fn(co, s, yb, t0, w)

    def conv_layer_pass(wkey, bt, n_co, taps, xin_c, seg_in, Cs, out_fn, ytag,
                        ydt=F32):
        """conv4/conv5 path (n_ci=4): two ci-pair passes, per-sample psum tiles
        kept alive across both passes (only 2 weight tags resident)."""
        halo_w = taps * 128
        for co in range(n_co):
            pss = [psum_conv.tile([128, Cs[s]], F32, tag=f"cp{s}", bufs=1,
                                  name=f"cp{s}") for s in range(NS)]
            for ph in range(2):
                wts = []
                for q in range(2):
                    wt = wconv.tile([128, halo_w], F16, tag=f"wc{q}", name=f"wc{q}")
                    nc.sync.dma_start(wt[:], d[wkey][co * 4 + ph * 2 + q, :, :])
                    wts.append(wt)
                for s in range(NS):
                    for q in range(2):
                        ci = ph * 2 + q
                        for tap in range(taps):
                            nc.tensor.matmul(
                                pss[s][:], wts[q][:, tap * 128:(tap + 1) * 128],
                                xin_c(ci)[:, seg_in[s] + tap: seg_in[s] + tap + Cs[s]],
                                start=(ci == 0 and tap == 0),
                                stop=(ci == 3 and tap == taps - 1))
                    step()
            for s in range(NS):
                yb = ystream.tile([128, Cs[s] + 2], ydt, tag=ytag, name=ytag)
                nc.vector.memset(yb[:, 0:1], NEG)
                nc.vector.memset(yb[:, Cs[s] + 1:Cs[s] + 2], NEG)
                nc.scalar.activation(yb[:, 1: 1 + Cs[s]], pss[s][:],
                                     AF.Relu, bias=bt[:, co:co + 1])
                step()
                out_fn(co, s, yb)

    # ---------------- conv2: 128 -> 256, k=11 ----------------
    def out2(co, s, yb, t0, w):
        maxpool_into(x3c(co)[:, seg3[s] + 8 + t0 // 2: seg3[s] + 8 + (t0 + w) // 2],
                     yb, w // 2, "mp2", F16)

    conv_layer("w2", b2t, 2, 1, 11, lambda ci: X2, seg2, Y2, out2, "y2", F16)

    # ---------------- conv3: 256 -> 512, k=17 ----------------
    X4 = acts.tile([128, slot1_w], F16, tag="slot1", name="X4")
    zero_halos(X4, seg4, V4, 8, 4, x4tot)

    def out3(co, s, yb, t0, w):
        maxpool_into(x4c(co)[:, seg4[s] + 8 + t0 // 2: seg4[s] + 8 + (t0 + w) // 2],
                     yb, w // 2, "mp3", F16)

    conv_layer("w3", b3t, 4, 2, 17, x3c, seg3, Y3, out3, "y3", F16)

    # ---------------- conv4: 512 -> 512, k=17 ----------------
    X5 = acts.tile([128, slot2_w], F16, tag="slot2", name="X5")
    zero_halos(X5, seg5, V5, 8, 4, x5tot)

    def out4(co, s, yb):
        maxpool_into(x5c(co)[:, seg5[s] + 8: seg5[s] + 8 + Y4[s] // 2],
                     yb, Y4[s] // 2, "mp4", F16)

    conv_layer_pass("w4", b4t, 4, 17, x4c, seg4, Y4, out4, "y4", ydt=F16)

    # ---------------- conv5: 512 -> 1024, k=17, + masked mean -------------
    # The gua f-linear accumulates incrementally in a persistent PSUM group
    # as each conv5 co-chunk's masked mean completes.
    xTg32 = geu_sb.tile([128, 8 * NS], F32)
    xTgh = geu_sb.tile([128, 8 * NS], F16)
    ones_row_h4 = consts.tile([1, NS], F16)
    nc.vector.memset(ones_row_h4[:], 1.0)
    agps = psum_geu.tile([128, 8 * NS], F32, tag="gps", name="agps")
    browf = small.tile([1, 1024], F16, tag="browa", name="browf")
    nc.gpsimd.dma_start(browf[:], d["guafb"][0:1, :])

    def out5(co, s, yb):
        maxpool_into(ac(co)[:, sega[s]: sega[s] + P[s]], yb, P[s], "mp5", F32)
        scr = ystream.tile([128, 128], F32, tag="mmean")
        nc.vector.scalar_tensor_tensor(
            scr[:, 0:P[s]], ac(co)[:, sega[s]: sega[s] + P[s]], 1.0,
            mbs[s][:, 0:P[s]], ALU.mult, ALU.mult,
            accum_out=xTg32[:, co * NS + s: co * NS + s + 1])
        if s == NS - 1:
            nc.vector.tensor_copy(xTgh[:, co * NS:(co + 1) * NS],
                                  xTg32[:, co * NS:(co + 1) * NS])
            wt = guaf_box[co // 2]
            off = (co % 2) * 1024
            for m in range(8):
                nc.tensor.matmul(
                    agps[:, m * NS:(m + 1) * NS],
                    wt[:, off + m * 128: off + (m + 1) * 128],
                    xTgh[:, co * NS:(co + 1) * NS],
                    start=(co == 0 and m == 0), stop=False)
            if co == 1:
                gua_chunk_dma(guaf_box, "guaf", 2)
            elif co == 3:
                gua_chunk_dma(guaf_box, "guaf", 3)
                gua_chunk_dma(guac_box, "guac", 0)
                gua_chunk_dma(guac_box, "guac", 1)
            elif co == 5:
                gua_chunk_dma(guac_box, "guac", 2)
                gua_chunk_dma(guac_box, "guac", 3)

    conv_layer_pass("w5", b5t, 8, 17, x5c, seg5, Y5, out5, "y5")

    flush()

    # ---------------- audio GEU tail: close guaf, run guac ----------------
    for m in range(8):
        nc.tensor.matmul(agps[:, m * NS:(m + 1) * NS],
                         browf[0:1, m * 128:(m + 1) * 128], ones_row_h4[:],
                         start=False, stop=(m == 7))
    x1aT = geu_sb.tile([128, 8 * NS], F16, name="x1aT")
    nc.scalar.copy(x1aT[:], agps[:])

    browc = small.tile([1, 1024], F16, tag="browa", name="browc")
    nc.sync.dma_start(browc[:], d["guacb"][0:1, :])
    ps2 = psum_geu.tile([128, 8 * NS], F32, tag="gps", name="acps")
    for m in range(8):
        wt = guac_box[m // 2]
        off = (m % 2) * 1024
        for k in range(8):
            nc.tensor.matmul(ps2[:, m * NS:(m + 1) * NS],
                             wt[:, off + k * 128: off + (k + 1) * 128],
                             x1aT[:, k * NS:(k + 1) * NS],
                             start=(k == 0), stop=False)
        nc.tensor.matmul(ps2[:, m * NS:(m + 1) * NS],
                         browc[0:1, m * 128:(m + 1) * 128], ones_row_h4[:],
                         start=False, stop=True)
    g1aT = geu_sb.tile([128, 8 * NS], F32, name="g1aT")
    nc.scalar.copy(g1aT[:], ps2[:])
    sgaT = geu_sb.tile([128, 8 * NS], F16, name="sgaT")
    nc.scalar.activation(sgaT[:], g1aT[:], AF.Sigmoid)
    x2aT = geu_sb.tile([128, 8 * NS], F16, name="x2aT")
    nc.vector.tensor_tensor(x2aT[:], x1aT[:], sgaT[:], ALU.mult)
    sq2 = ystream.tile([128, 8 * NS], F16, tag="vsq", name="sq2")
    nc.vector.tensor_tensor(sq2[:], x2aT[:], x2aT[:], ALU.mult)
    ssa_ps = psum_tp.tile([1, 8 * NS], F32, tag="tpp", name="ssa_ps")
    nc.tensor.matmul(ssa_ps[:], ones_r1[:], sq2[:], start=True, stop=True)
    ssa = small.tile([1, NS], F32, name="ssa")
    nc.vector.reduce_sum(ssa[:], ssa_ps[:].rearrange("one (k j) -> one j k", j=NS),
                         axis=mybir.AxisListType.X, opt_input=False)
    nc.vector.tensor_scalar_max(ssa[:], ssa[:], 1e-24)
    ssaq = small.tile([1, NS], F32, name="ssaq")
    nc.scalar.activation(ssaq[:], ssa[:], AF.Sqrt)
    invarow = small.tile([1, 8 * NS], F32, name="invarow")
    nc.vector.reciprocal(invarow[:, 0:NS], ssaq[:])
    for r in (1, 2, 4):
        nc.vector.tensor_copy(invarow[:, r * NS:2 * r * NS], invarow[:, 0:r * NS])
    inva_b = consts.tile([128, 8 * NS], F32, name="inva_b")
    nc.gpsimd.partition_broadcast(inva_b[:], invarow[:])
    gaT = geu_sb.tile([128, 8 * NS], F16, name="gaT")
    nc.vector.tensor_tensor(gaT[:], x2aT[:], inva_b[:], ALU.mult)

    # ---------------- local projection: out2[s, :] for own samples --------
    browp = small.tile([1, 4096], F16, tag="browp", name="browp")
    nc.scalar.dma_start(browp[:], d["projb"][0:1, :])
    for n in range(8):
        pool_n = psum_geu if n % 2 == 0 else psum_tp
        psp = pool_n.tile([NS, 512], F32, tag="gps" if n % 2 == 0 else "tpp",
                          name="pgps")
        for k in range(8):
            nc.tensor.matmul(psp[:], gaT[:, k * NS:(k + 1) * NS],
                             get_projw()[:, k * 4096 + n * 512: k * 4096 + (n + 1) * 512],
                             start=(k == 0), stop=False)
        nc.tensor.matmul(psp[:], ones_row_h4[:],
                         browp[:, n * 512:(n + 1) * 512], start=False, stop=True)
        ot_sb = ystream.tile([NS, 512], F32, tag="geu_tmp", name="ot_sb")
        nc.scalar.copy(ot_sb[:], psp[:])
        nc.sync.dma_start(d["out2"][:, n * 512:(n + 1) * 512], ot_sb[:])

    tv_out_epilogue()


def build(P):
    nc = bacc.Bacc()
    d = declare_io(nc, P)
    with tile.TileContext(nc) as tc:
        with ExitStack() as ctx:
            emit(ctx, tc, d, P)
    nc.compile()
    return nc


# ---------------------------------------------------------------------------
# host-side planning + data prep
# ---------------------------------------------------------------------------
def plan_from_inputs(inputs):
    """sample -> (core, slot) assignment and compiled slot lengths P."""
    nfr = np.asarray(inputs["audio_STFT_nframes"]).astype(np.int64)
    nf = np.maximum(1, nfr // 16)
    order = np.argsort(-nf, kind="stable")
    P = []
    for j in range(NS):
        Pa = int(nf[order[j * NC:(j + 1) * NC]].max())
        P.append(min(128, ((Pa + 3) // 4) * 4))
    return order, tuple(P)


def prep_shared(inp):
    """Replicated weights, host-transposed/cast."""
    f32, f16 = np.float32, np.float16
    w = {}
    bn_scale = (np.asarray(inp["bn_g"])[0] /
                np.sqrt(np.float32(1.0) + np.float32(1e-5))).astype(f32)
    c1 = np.asarray(inp["c1w"])[:, 0, :, 0].astype(f32)   # (128, 40)
    w["w1T"] = np.ascontiguousarray((c1 * bn_scale).T.astype(f16))
    w["b1"] = np.ascontiguousarray(
        (np.asarray(inp["c1b"]) + np.asarray(inp["bn_b"])[0] * c1.sum(1)).astype(f32)[:, None])

    def conv_w(cw, coutp, cinp, taps):
        cw = np.asarray(cw)
        ci = cw.shape[1]
        cin = ci // cinp
        a = cw[:, :, 0, :].astype(f32)                    # (Cout, Cin, taps)
        a = a.reshape(coutp, 128, cinp, cin, taps)
        a = a.transpose(0, 2, 3, 4, 1)                    # coutp, cinp, cin, tap, cout
        return np.ascontiguousarray(a.reshape(coutp * cinp, cin, taps * 128).astype(f16))

    def bias_t(b, coutp):
        return np.ascontiguousarray(np.asarray(b).astype(f32).reshape(coutp, 128).T)

    w["w2"] = conv_w(inp["c2w"], 2, 1, 11); w["b2"] = bias_t(inp["c2b"], 2)
    w["w3"] = conv_w(inp["c3w"], 4, 2, 17); w["b3"] = bias_t(inp["c3b"], 4)
    w["w4"] = conv_w(inp["c4w"], 4, 4, 17); w["b4"] = bias_t(inp["c4b"], 4)
    w["w5"] = conv_w(inp["c5w"], 8, 4, 17); w["b5"] = bias_t(inp["c5b"], 8)

    w["tpT"] = np.ascontiguousarray(np.asarray(inp["tp_w"]).astype(f32).T.astype(f16))
    w["tpb"] = np.ascontiguousarray(np.asarray(inp["tp_b"]).astype(f32).reshape(32, 128).T)

    for nm, src in (("guaf", "gua_fw"), ("guac", "gua_cw")):
        wT = np.asarray(inp[src]).astype(f32).T.astype(f16)   # (1024 k, 1024 n)
        if nm == "guaf":
            # k-major: chunk j holds k-chunks 2j, 2j+1 (incremental accum
            # inside conv5 epilogues)
            a = wT.reshape(8, 128, 1024).transpose(1, 0, 2)   # p, k, n
        else:
            # m-major: chunk j holds m-chunks 2j, 2j+1 (tail, all k resident)
            a = wT.reshape(8, 128, 8, 128).transpose(1, 2, 0, 3)  # p, m, k, mp
        w[nm + "T"] = np.ascontiguousarray(a.reshape(128, 8 * 1024))
        w[nm + "b"] = np.ascontiguousarray(
            np.asarray(inp[src.replace("w", "b")]).astype(f16)[None, :])

    wT = np.asarray(inp["proj_w"]).astype(f32).T.astype(f16)  # (1024, 4096)
    a = wT.reshape(8, 128, 4096).transpose(1, 0, 2)
    w["projT"] = np.ascontiguousarray(a.reshape(128, 8 * 4096))
    w["projb"] = np.ascontiguousarray(np.asarray(inp["proj_b"]).astype(f16)[None, :])
    return w


def prep_core_inputs(inp, w, order, P, core):
    """Per-core input map: local samples + this core's GEU weight slices."""
    f16 = np.float16
    S = derive_sizes(P)
    m = dict(w)
    samples = [int(order[NC * j + core]) for j in range(NS)]

    audio = np.asarray(inp["audio"]).astype(np.float32)
    m["aT"] = np.ascontiguousarray(
        np.concatenate([audio[samples[j], :, 0:S["V2"][j]] for j in range(NS)],
                       axis=1).astype(f16))
    m["tT"] = np.ascontiguousarray(
        np.asarray(inp["text"])[samples].astype(f16).transpose(2, 0, 1).reshape(300, NS * 30))
    m["vT"] = np.ascontiguousarray(
        np.asarray(inp["video"])[samples].astype(f16).transpose(2, 0, 1).reshape(4096, NS * 16))
    nfr = np.asarray(inp["audio_STFT_nframes"]).astype(np.int64)[samples]
    m["nf"] = np.ascontiguousarray(np.maximum(1, nfr // 16).astype(np.int32)[:, None])

    sl = slice(512 * core, 512 * (core + 1))
    for nm, src in (("gutf", "gut_fw"), ("gutc", "gut_cw"),
                    ("guvf", "guv_fw"), ("guvc", "guv_cw")):
        wT = np.asarray(inp[src]).astype(np.float32).T[:, sl].astype(f16)  # (4096, 512)
        a = wT.reshape(32, 128, 512).transpose(1, 0, 2)
        m[nm + "T"] = np.ascontiguousarray(a.reshape(128, 32 * 512))
        m[nm + "b"] = np.ascontiguousarray(
            np.asarray(inp[src.replace("w", "b")]).astype(f16)[None, sl])
    return m


def assemble_output(results, order):
    """results[c]: {"out": [2, 32, 512] (all samples, this core's 512 cols),
    "out2": [NS, 4096] (this core's samples, all cols)}."""
    full = np.empty((3, NB, 4096), np.float32)
    inv = np.empty(NB, np.int64)
    for p in range(NB):
        c, j = p // NS, p % NS
        inv[p] = order[NC * j + c]
    for c2 in range(NC):
        full[0:2, inv, 512 * c2:512 * (c2 + 1)] = results[c2]["out"]
        for j in range(NS):
            full[2, order[NC * j + c2], :] = results[c2]["out2"][j]
    return full


# ---------------------------------------------------------------------------
# public entry point
# ---------------------------------------------------------------------------
_NC_CACHE = {}


def _get_nc(P=None):
    if P is None:
        assert _NC_CACHE, "call kernel() or prepare() first"
        return next(iter(_NC_CACHE.values()))
    if P not in _NC_CACHE:
        _NC_CACHE[P] = build(P)
    return _NC_CACHE[P]


def prepare(inputs):
    order, P = plan_from_inputs(inputs)
    nc = _get_nc(P)
    w = prep_shared(inputs)
    in_maps = [prep_core_inputs(inputs, w, order, P, c) for c in range(NC)]
    return nc, in_maps, order, P


def kernel(**inputs):
    from concourse.bass_utils import run_bass_kernel_spmd

    nc, in_maps, order, P = prepare(inputs)
    res = run_bass_kernel_spmd(nc, in_maps, core_ids=list(range(NC)))
    return assemble_output([res.results[c] for c in range(NC)], order)
